# revision 10
# baseline (speedup 1.0000x reference)
"""GCN layer (gather -> mean-aggregate -> linear -> relu) on 8 TRN2 NeuronCores.

Strategy:
- Nodes/outputs sharded by destination (12500 per core); edges partitioned by
  destination core; h and the 64x64 weight replicated.
- Reorder: out = relu(mask * (mean_agg(h) @ W.T + b)); each core gathers raw
  h rows for its edges (dma_gather, int16 idxs, 4 SWDGE queues), segment-sums
  them per 128-dst block via one-hot selection matmuls accumulating in PSUM,
  then projects per block. Degree vector is host-side sharding metadata.
- Edges are grouped (dst-block, src-group of 32768) with all-core-uniform
  static capacities (multiple of 16; idx-0 pads killed by -1 dst slots).
  Two consecutive dst blocks form a super-block: for each src group the two
  cells are packed into shared ~1024-idx gather instructions; a 128-edge
  chunk that spans the block boundary gets one one-hot matmul ("touch") per
  block. PSUM holds a 4-deep ring of block accumulators.
"""

import numpy as np
from contextlib import ExitStack

N_NODES = 100000
N_EDGES = 1600000
D = 64
NCORES = 8
NPC = N_NODES // NCORES          # dsts per core
NB = (NPC + 127) // 128          # dst blocks per core
GS = 32768                       # src group size (int16 index range)
NG = (N_NODES + GS - 1) // GS    # src groups
MAX_IDX = 1024                   # per dma_gather instruction
NBUF = 8                         # gather buffers (8 chunks each)
SELR = 24                        # sel tile ring (touches)


def _round16(x):
    return (x + 15) & ~15


def _host_partition(edge_src, edge_dst):
    core = edge_dst // NPC
    per_core = []
    counts = np.zeros((NCORES, NB, NG), np.int64)
    for c in range(NCORES):
        m = np.nonzero(core == c)[0]
        src_c = edge_src[m]
        dst_c = edge_dst[m] - c * NPC
        blk = dst_c >> 7
        grp = src_c >> 15
        order = np.lexsort((grp, blk))
        src_c = src_c[order]
        dst_c = dst_c[order]
        cell = (dst_c >> 7) * NG + (src_c >> 15)
        counts[c] = np.bincount(cell, minlength=NB * NG).reshape(NB, NG)
        per_core.append((src_c, dst_c))

    caps = np.zeros((NB, NG), np.int64)
    for B in range(NB):
        for g in range(NG):
            caps[B, g] = _round16(int(counts[:, B, g].max()))

    NS = (NB + 1) // 2
    # Build instruction plan. Each instr: (g, n, seg_ofs, touches)
    # touches: list of (block, start, stop, chunk_in_instr) in order.
    # Segment for (S, g): cap(2S, g) slots for block 2S then cap(2S+1, g).
    plan = []                     # dicts
    touch_count_of_block = np.zeros(NB, np.int64)
    # first pass: count touches per block to set start/stop flags later
    seg_meta = []                 # (S, g, L, cap0)
    for S in range(NS):
        b0 = 2 * S
        b1 = 2 * S + 1 if 2 * S + 1 < NB else None
        for g in range(NG):
            cap0 = int(caps[b0, g])
            cap1 = int(caps[b1, g]) if b1 is not None else 0
            L = cap0 + cap1
            if L == 0:
                continue
            seg_meta.append((S, g, L, cap0))

    # build instrs + touches
    touches_all = []              # global touch list: (block)
    for (S, g, L, cap0) in seg_meta:
        b0 = 2 * S
        b1 = 2 * S + 1
        qofs = 0
        while qofs < L:
            n = min(MAX_IDX, L - qofs)
            kb = (n + 127) // 128
            touches = []
            for ch in range(kb):
                lo = qofs + ch * 128
                hi = min(qofs + ch * 128 + 128, L)
                if lo < cap0:
                    touches.append([b0, ch])
                if hi > cap0:
                    touches.append([b1, ch])
            plan.append(dict(S=S, g=g, n=n, qofs=qofs, cap0=cap0,
                             touches=touches))
            for t in touches:
                touches_all.append(t[0])
            qofs += n

    # start/stop flags per touch
    seen = np.zeros(NB, np.int64)
    tot = np.bincount(touches_all, minlength=NB)
    ti = 0
    last_instr_of_block = np.zeros(NB, np.int64)
    touch_end = []
    for i, ins in enumerate(plan):
        for t in ins["touches"]:
            b = t[0]
            seen[b] += 1
            t.append(seen[b] == 1)          # start
            t.append(seen[b] == tot[b])     # stop
            if seen[b] == tot[b]:
                last_instr_of_block[b] = i
            ti += 1
        touch_end.append(ti)
    n_touch = ti
    idx_w = sum(ins["n"] // 16 for ins in plan)

    # per-core data arrays
    idx_arrs, dv_arrs, deg_arrs = [], [], []
    for c in range(NCORES):
        src_c, dst_c = per_core[c]
        deg = np.bincount(dst_c, minlength=NB * 128).astype(np.float32)
        rdeg = 1.0 / np.maximum(deg, 1.0)
        ind = np.minimum(deg, 1.0)
        deg_arrs.append((rdeg.reshape(NB, 128).T.copy(),
                         ind.reshape(NB, 128).T.copy()))
        cnt = counts[c]
        cell_starts = np.zeros(NB * NG + 1, np.int64)
        np.cumsum(cnt.reshape(-1), out=cell_starts[1:])

        # segment content per (S, g): idx values + dst-in-block values laid
        # out over the segment positions; -1 dst marks pads.
        idx16 = np.zeros((128, idx_w), np.int16)
        dv = np.full((128, n_touch), -1.0, np.float32)
        seg_cache = {}
        for (S, g, L, cap0) in seg_meta:
            vals = np.zeros(L, np.int16)
            dsts = np.full(L, -1.0, np.float32)
            for half, b in enumerate((2 * S, 2 * S + 1)):
                if b >= NB:
                    continue
                base = 0 if half == 0 else cap0
                ci = b * NG + g
                k = int(cnt[b, g])
                s0 = int(cell_starts[ci])
                if k > 0:
                    vals[base:base + k] = (src_c[s0:s0 + k] & (GS - 1)).astype(np.int16)
                    dsts[base:base + k] = (dst_c[s0:s0 + k] & 127).astype(np.float32)
            seg_cache[(S, g)] = (vals, dsts)

        wofs = 0
        tcol = 0
        for ins in plan:
            S, g, n, qofs, cap0 = ins["S"], ins["g"], ins["n"], ins["qofs"], ins["cap0"]
            vals, dsts = seg_cache[(S, g)]
            flat = np.zeros(_round16(n), np.int16)
            flat[:n] = vals[qofs:qofs + n]
            w = n // 16
            idx16[:, wofs:wofs + w] = np.tile(flat.reshape(w, 16).T, (8, 1))
            wofs += w
            for (b, ch, _st, _sp) in ins["touches"]:
                lo = qofs + ch * 128
                hi = min(lo + 128, qofs + n)
                pos = np.arange(lo, hi)
                slot = pos - (qofs + ch * 128)
                # this touch's block occupies positions [..] of the segment
                if b == 2 * S:
                    m = pos < cap0
                else:
                    m = pos >= cap0
                dsel = np.where(m, dsts[lo:hi], -1.0)
                dv[slot, tcol] = dsel
                tcol += 1
        idx_arrs.append(idx16)
        dv_arrs.append(dv)

    meta = dict(plan=plan, touch_end=touch_end, n_touch=n_touch, idx_w=idx_w,
                last_instr_of_block=last_instr_of_block)
    return meta, idx_arrs, dv_arrs, deg_arrs


def _build_nc(meta):
    import concourse.bacc as bacc
    import concourse.mybir as mybir
    from concourse.library_config import mlp
    from concourse._compat import get_trn_type

    f32 = mybir.dt.float32
    i16 = mybir.dt.int16
    glens = [min(GS, N_NODES - g * GS) for g in range(NG)]
    plan = meta["plan"]
    touch_end = meta["touch_end"]
    n_touch = meta["n_touch"]
    idx_w = meta["idx_w"]
    last_instr_of_block = meta["last_instr_of_block"]

    nc = bacc.Bacc(get_trn_type() or "TRN2", debug=True, num_swdge_queues=4)
    h_d = nc.declare_dram_parameter("h", [N_NODES, D], f32, isOutput=False)
    idx_d = nc.declare_dram_parameter("idx", [128, idx_w], i16, isOutput=False)
    dv_d = nc.declare_dram_parameter("dv", [128, n_touch], f32, isOutput=False)
    cst_d = nc.declare_dram_parameter("cst", [128, 257], f32, isOutput=False)
    wa_d = nc.declare_dram_parameter("wa", [65, D], f32, isOutput=False)
    rdeg_d = nc.declare_dram_parameter("rdeg", [128, NB], f32, isOutput=False)
    ind_d = nc.declare_dram_parameter("ind", [128, NB], f32, isOutput=False)
    out_d = nc.declare_dram_parameter("out", [NB * 128, D], f32, isOutput=True)

    with ExitStack() as st:
        e = st.enter_context
        idx_sb = e(nc.sbuf_tensor("idx_sb", [128, idx_w], i16))
        dv_sb = e(nc.sbuf_tensor("dv_sb", [128, n_touch], f32))
        cst_sb = e(nc.sbuf_tensor("cst_sb", [128, 257], f32))
        wa_sb = e(nc.sbuf_tensor("wa_sb", [65, D], f32))
        rdeg_sb = e(nc.sbuf_tensor("rdeg_sb", [128, NB], f32))
        ind_sb = e(nc.sbuf_tensor("ind_sb", [128, NB], f32))
        gbuf = [e(nc.sbuf_tensor(f"gbuf{i}", [128, 8 * D], f32)) for i in range(NBUF)]
        sel = [e(nc.sbuf_tensor(f"sel{i}", [128, 128], f32)) for i in range(SELR)]
        agg = [e(nc.sbuf_tensor(f"agg{i}", [128, 65], f32)) for i in range(2)]
        aggT = [e(nc.sbuf_tensor(f"aggT{i}", [65, 128], f32)) for i in range(2)]
        ysb = [e(nc.sbuf_tensor(f"ysb{i}", [64, 128], f32)) for i in range(2)]
        otile = [e(nc.sbuf_tensor(f"otile{i}", [128, D], f32)) for i in range(2)]

        acc = [e(nc.psum_tensor(f"acc{i}", [128, D], f32)) for i in range(4)]
        pt1 = [e(nc.psum_tensor("pt1", [65, 128], f32))] * 2
        pmw = [e(nc.psum_tensor("pmw", [64, 128], f32))] * 2
        pt2 = [e(nc.psum_tensor("pt2", [128, D], f32))] * 2

        in_s = e(nc.semaphore("in_s"))
        g_s = [e(nc.semaphore(f"g_s{i}")) for i in range(NBUF)]
        pe_s = e(nc.semaphore("pe_s"))      # instrs consumed by PE
        sel_s = e(nc.semaphore("sel_s"))    # touches built by DVE
        dep_s = e(nc.semaphore("dep_s"))    # blocks aggregated by DVE
        pt1_s = e(nc.semaphore("pt1_s"))
        dt1_s = e(nc.semaphore("dt1_s"))
        pmw_s = e(nc.semaphore("pmw_s"))
        act_s = e(nc.semaphore("act_s"))
        pt2_s = e(nc.semaphore("pt2_s"))
        dvo_s = e(nc.semaphore("dvo_s"))
        out_s = e(nc.semaphore("out_s"))
        ms_s = e(nc.semaphore("ms_s"))
        block = e(nc.Block())

        iota_ap = lambda: cst_sb[:, 0:128]
        ident_ap = lambda: cst_sb[:, 128:256]

        # group instrs by super-block for epilogue placement
        instr_super = [ins["S"] for ins in plan]
        NS = (NB + 1) // 2

        @block.gpsimd
        def _(eng):
            eng.load_library(mlp)
            eng.dma_start(out=idx_sb[:], in_=idx_d[:]).then_inc(in_s, 16)
            eng.dma_start(out=dv_sb[:], in_=dv_d[:]).then_inc(in_s, 16)
            eng.dma_start(out=cst_sb[:], in_=cst_d[:]).then_inc(in_s, 16)
            eng.dma_start(out=wa_sb[:], in_=wa_d[:]).then_inc(in_s, 16)
            eng.dma_start(out=rdeg_sb[:], in_=rdeg_d[:]).then_inc(in_s, 16)
            eng.dma_start(out=ind_sb[:], in_=ind_d[:]).then_inc(in_s, 16)
            eng.wait_ge(in_s, 96)
            for bb in range(NBUF):
                eng.memset(gbuf[bb][:], 0.0).then_inc(ms_s, 1)
            eng.wait_ge(ms_s, NBUF)
            wofs = 0
            for i, ins in enumerate(plan):
                n, g = ins["n"], ins["g"]
                if i >= NBUF:
                    eng.wait_ge(pe_s, i - NBUF + 1)
                kb = (n + 127) // 128
                eng.dma_gather(
                    out_ap=gbuf[i % NBUF][:, : kb * D].rearrange(
                        "p (k d) -> p k d", d=D
                    ),
                    in_ap=h_d[g * GS : g * GS + glens[g], :],
                    idxs_ap=idx_sb[:, wofs : wofs + n // 16],
                    num_idxs=n,
                    num_idxs_reg=n,
                    elem_size=D,
                    queue_num=i % 4,
                ).then_inc(g_s[i % NBUF], 16)
                wofs += n // 16

        @block.tensor
        def _(eng):
            eng.wait_ge(in_s, 96)

            def pe_ep(b):
                p = b % 2
                eng.wait_ge(dep_s, b + 1)
                if b >= 1:
                    eng.wait_ge(dt1_s, b)
                eng.matmul(
                    out=pt1[p][:], lhsT=agg[p][:], rhs=ident_ap(),
                    is_transpose=True,
                ).then_inc(pt1_s, 1)
                eng.wait_ge(dt1_s, b + 1)
                if b >= 1:
                    eng.wait_ge(act_s, b)
                eng.matmul(
                    out=pmw[p][:], lhsT=wa_sb[:], rhs=aggT[p][:],
                    start=True, stop=True,
                ).then_inc(pmw_s, 1)
                eng.wait_ge(act_s, b + 1)
                if b >= 1:
                    eng.wait_ge(dvo_s, b)
                eng.matmul(
                    out=pt2[p][:], lhsT=ysb[p][:], rhs=ident_ap()[:64, :64],
                    is_transpose=True,
                ).then_inc(pt2_s, 1)

            tglob = 0
            prev_S = 0
            for i, ins in enumerate(plan):
                S = ins["S"]
                if S != prev_S:
                    for b in (2 * (S - 1), 2 * (S - 1) + 1):
                        if b < NB:
                            pe_ep(b)
                    prev_S = S
                eng.wait_ge(g_s[i % NBUF], 16 * (i // NBUF + 1))
                eng.wait_ge(sel_s, touch_end[i])
                nt = len(ins["touches"])
                for k, (b, ch, tstart, tstop) in enumerate(ins["touches"]):
                    if tstart and b >= 4:
                        eng.wait_ge(dep_s, b - 3)
                    mm = eng.matmul(
                        out=acc[b % 4][:],
                        lhsT=sel[tglob % SELR][:],
                        rhs=gbuf[i % NBUF][:, ch * D : (ch + 1) * D],
                        start=tstart, stop=tstop,
                    )
                    if k == nt - 1:
                        mm.then_inc(pe_s, 1)
                    tglob += 1
            for b in (2 * (NS - 1), 2 * (NS - 1) + 1):
                if b < NB:
                    pe_ep(b)

        @block.vector
        def _(eng):
            import concourse.mybir as mb
            eng.wait_ge(in_s, 96)

            def dve_ep(b):
                p = b % 2
                eng.wait_ge(pe_s, int(last_instr_of_block[b]) + 1)
                if b >= 1:
                    eng.wait_ge(pt1_s, b)
                eng.tensor_copy(out=agg[p][:, 64:65], in_=ind_sb[:, b : b + 1])
                eng.tensor_scalar(
                    out=agg[p][:, 0:64], in0=acc[b % 4][:],
                    scalar1=rdeg_sb[:, b : b + 1], scalar2=None,
                    op0=mb.AluOpType.mult,
                ).then_inc(dep_s, 1)
                eng.wait_ge(pt1_s, b + 1)
                if b >= 1:
                    eng.wait_ge(pmw_s, b)
                eng.tensor_copy(out=aggT[p][:], in_=pt1[p][:]).then_inc(dt1_s, 1)
                eng.wait_ge(pt2_s, b + 1)
                if b >= 1:
                    eng.wait_ge(out_s, 16 * b)
                eng.tensor_copy(out=otile[p][:], in_=pt2[p][:]).then_inc(dvo_s, 1)

            tglob = 0
            prev_S = 0
            for i, ins in enumerate(plan):
                S = ins["S"]
                if S != prev_S:
                    for b in (2 * (S - 1), 2 * (S - 1) + 1):
                        if b < NB:
                            dve_ep(b)
                    prev_S = S
                if i >= 2:
                    eng.wait_ge(pe_s, i - 1)
                for (b, ch, _st, _sp) in ins["touches"]:
                    eng.tensor_tensor(
                        out=sel[tglob % SELR][:],
                        in0=dv_sb[:, tglob : tglob + 1].to_broadcast([128, 128]),
                        in1=iota_ap(),
                        op=mb.AluOpType.is_equal,
                    ).then_inc(sel_s, 1)
                    tglob += 1
            for b in (2 * (NS - 1), 2 * (NS - 1) + 1):
                if b < NB:
                    dve_ep(b)

        @block.scalar
        def _(eng):
            import concourse.mybir as mb
            for b in range(NB):
                eng.wait_ge(pmw_s, b + 1)
                if b >= 1:
                    eng.wait_ge(pt2_s, b)
                eng.activation(
                    out=ysb[b % 2][:], in_=pmw[b % 2][:],
                    func=mb.ActivationFunctionType.Relu,
                ).then_inc(act_s, 1)

        @block.sync
        def _(eng):
            for b in range(NB):
                eng.wait_ge(dvo_s, b + 1)
                eng.dma_start(
                    out=out_d[b * 128 : (b + 1) * 128, :], in_=otile[b % 2][:]
                ).then_inc(out_s, 16)
            eng.wait_ge(out_s, 16 * NB)

    nc.compile()
    return nc


def _host_inputs(h, W, b, idx_arrs, dv_arrs, deg_arrs):
    cst = np.zeros((128, 257), np.float32)
    cst[:, 0:128] = np.arange(128, dtype=np.float32)[None, :]
    cst[:, 128:256] = np.eye(128, dtype=np.float32)
    cst[:, 256] = 1.0
    wa = np.concatenate([W.T.astype(np.float32), b.astype(np.float32)[None, :]], axis=0)
    in_maps = []
    for c in range(NCORES):
        in_maps.append({
            "h": np.ascontiguousarray(h.astype(np.float32)),
            "idx": idx_arrs[c],
            "dv": dv_arrs[c],
            "cst": cst,
            "wa": wa,
            "rdeg": deg_arrs[c][0],
            "ind": deg_arrs[c][1],
        })
    return in_maps


def kernel(h, edge_src, edge_dst, W, b):
    h = np.asarray(h, np.float32)
    edge_src = np.asarray(edge_src, np.int32)
    edge_dst = np.asarray(edge_dst, np.int32)
    W = np.asarray(W, np.float32)
    b = np.asarray(b, np.float32)

    from concourse.bass_utils import run_bass_kernel_spmd

    meta, idx_arrs, dv_arrs, deg_arrs = _host_partition(edge_src, edge_dst)
    nc = _build_nc(meta)
    in_maps = _host_inputs(h, W, b, idx_arrs, dv_arrs, deg_arrs)
    res = run_bass_kernel_spmd(nc, in_maps, list(range(NCORES)))
    out = np.concatenate(
        [res.results[c]["out"][:NPC] for c in range(NCORES)], axis=0
    )
    return out.astype(np.float32)
